# revision 58
# baseline (speedup 1.0000x reference)
"""OIM loss with circular queue — Trainium2 Bass kernel (8 NeuronCores).

Strategy (v8)
-------------
loss = mean_b [ M + log S_b - 30*cos(x_b, e_xe_b) ],
S_b = sum_{q good} exp(30*cos(x_b, e_q) - M)  over the post-update queue.

Integer bookkeeping (queue update, target slots) and data layout run on the
host: x is normalized, quantized to fp8(e4m3), shipped d-major (logits
matmuls) and b-major (per-pid means); emb_cq is shipped d-major fp8 with
dead columns (invalidated or window-overwritten slots) zeroed.  A zeroed
column contributes exactly exp(-M) (or the bit-trick equivalent) to S, which
the host subtracts in closed form — no on-device masking anywhere.

Device work per core (Q sharded, QS=2048 columns each):
  - per-pid means via fp8 DoubleRow matmuls (K=256/instr, 0.5 cyc/row),
    normalized (Quake rsqrt) and transposed into 256 fresh "window"
    columns (uembT8), which are also DMA'd back so the host can compute
    the target cosines itself (bit-exact fp8 gather-dot)
  - logits + exp + row-sum in two orientations to balance the engines:
      A-side [b-part, q-free]: psum -> ACT exp with accum_out (sum free)
      B-side [q-part, b-free]: psum -> exp -> PE ones-matmul accumulator;
        the window columns ride the B-side with a separate accumulator
        (they are identical on all cores, so the host counts them once)
    exp is either exact (ACT, bf16 out) or a bf16 Schraudolph bit-trick
    (DVE tensor_scalar f32->int16, bitcast bf16), assigned per unit
Host combines: S = A-sums + B-sums + window-B-sums(core 0) - per-row
zero-column corrections; loss assembled in f64.
"""

import os
import sys

import numpy as np

for _p in ("/opt/trn_rl_repo", "/root/.axon_site/_ro/trn_rl_repo"):
    if os.path.isdir(_p) and _p not in sys.path:
        sys.path.insert(0, _p)

B, D, Q, U = 4096, 512, 16384, 256
N_CORES = 8
QS = Q // N_CORES          # queue rows per core
OIM_SCALAR = 30.0
IGNORE = -1
MT = B // 128              # 32 b-tiles
KD = D // 128              # 4 contraction chunks of 128
KD2 = D // 256             # 2 double-row contraction chunks of 256
NC16 = B // 256            # 16 b-pair chunks for the means matmul

# ---- tuning knobs -------------------------------------------------------
QA = 1024                  # A-orientation raw columns (tail of the shard)
QB = QS - QA               # B-orientation raw columns (head of the shard)
BW = 512                   # B-side batch window
NBW = B // BW              # 8 windows
NQT = QB // 128            # raw q-tiles per B window
NUT = U // 128             # window q-tiles per B window
NT = NQT + NUT             # total q-tiles per B window

A16 = 128.0 / float(np.log(2.0))   # bf16 Schraudolph: bits = A16*z + SB16
SB16 = 127.0 * 128.0 - 7.0


def e_B(w, t):
    """True -> B unit (w, t) exp'd with the DVE bit-trick (t>=NQT are the
    window tiles)."""
    return ((w * NT + t) % 5) != 0


_PROG_CACHE = {}


def _build_program(M: float):
    """Emit + schedule + compile the (SPMD, identical on all cores) program."""
    import concourse.bacc as bacc
    import concourse.tile as tile
    from concourse import mybir
    from concourse.masks import make_identity

    f32 = mybir.dt.float32
    i32 = mybir.dt.int32
    f8 = mybir.dt.float8e4
    bf16 = mybir.dt.bfloat16
    AF = mybir.ActivationFunctionType
    OP = mybir.AluOpType
    DR = mybir.MatmulPerfMode.DoubleRow

    nc = bacc.Bacc("TRN2", target_bir_lowering=False, debug=False,
                   num_devices=N_CORES)

    embT8_d = nc.dram_tensor("embT8", [128, KD, QS], f8, kind="ExternalInput").ap()
    xnT8_d = nc.dram_tensor("xnT8", [128, KD, B], f8, kind="ExternalInput").ap()
    x8b_d = nc.dram_tensor("x8b", [128, NC16, 2, D], f8, kind="ExternalInput").ap()
    mask8_d = nc.dram_tensor("mask8", [128, NC16, 2, U], f8, kind="ExternalInput").ap()
    sume_d = nc.dram_tensor("sume", [128, MT], f32, kind="ExternalOutput").ap()
    srow_d = nc.dram_tensor("srow", [128, 6, BW], f32, kind="ExternalOutput").ap()
    uout_d = nc.dram_tensor("uout", [128, KD, U], f8, kind="ExternalOutput").ap()

    with tile.TileContext(nc) as tc:
        with (
            tc.tile_pool(name="singles", bufs=1) as singles,
            tc.tile_pool(name="eap", bufs=16) as eap,
            tc.tile_pool(name="scrp", bufs=4) as scrp,
            tc.tile_pool(name="small", bufs=4) as small,
        ):
            ident = singles.tile([128, 128], f32)
            make_identity(nc, ident)
            ones16 = singles.tile([128, 1], bf16)
            nc.vector.memset(ones16, 1.0)
            biasM = singles.tile([128, 1], f32)
            nc.vector.memset(biasM, -M)

            # resident inputs
            xnT8 = singles.tile([128, KD, B], f8)
            embT8 = singles.tile([128, KD, QS], f8)
            x8b = singles.tile([128, NC16, 2, D], f8)
            mask8 = singles.tile([128, NC16, 2, U], f8)

            # DMA order tuned for startup overlap: B units (w=0) need the
            # embT8 head + first 512 xnT8 columns; A(0) adds the embT8 tail.
            nc.sync.dma_start(out=embT8[:, :, :QB], in_=embT8_d[:, :, :QB])
            nc.sync.dma_start(out=xnT8[:, :, :512], in_=xnT8_d[:, :, :512])
            nc.sync.dma_start(out=embT8[:, :, QB:], in_=embT8_d[:, :, QB:])
            nc.sync.dma_start(out=xnT8[:, :, 512:2048], in_=xnT8_d[:, :, 512:2048])
            for cc in range(2):
                nc.sync.dma_start(out=x8b[:, 8 * cc:8 * cc + 8],
                                  in_=x8b_d[:, 8 * cc:8 * cc + 8])
                nc.sync.dma_start(out=mask8[:, 8 * cc:8 * cc + 8],
                                  in_=mask8_d[:, 8 * cc:8 * cc + 8])
            nc.sync.dma_start(out=xnT8[:, :, 2048:], in_=xnT8_d[:, :, 2048:])

            # device-computed residents
            uembT8 = singles.tile([128, KD, U], f8)     # normalized means, d-major
            uemb_n = singles.tile([128, 2, D], f32)     # normalized means, u-major
            uraw = singles.tile([128, 2, D], f32)       # raw masked sums
            ssb = singles.tile([128, MT], f32)          # A-side sums
            # B-side row sums: 4 packed banks (raw w0-3, raw w4-7, win w0-3,
            # win w4-7), real data on partitions 0/32/64/96
            srow_sb = singles.tile([128, 6, BW], f32)

            def emit_A(m):
                # raw A columns [QB, QS) in one 2-bank psum tile
                pa_t = psum_a.tile([128, QA], f32, tag="pa", name=f"pa{m}")
                for h in range(QA // 256):
                    for c in range(KD2):
                        nc.tensor.matmul(
                            pa_t[:, h * 256:(h + 1) * 256],
                            xnT8[:, 2 * c:2 * c + 2, m * 128:(m + 1) * 128],
                            embT8[:, 2 * c:2 * c + 2,
                                  QB + h * 256:QB + (h + 1) * 256],
                            start=(c == 0), stop=(c == KD2 - 1),
                            perf_mode=DR)
                nc.scalar.activation(out=pa_t, in_=pa_t, func=AF.Exp,
                                     bias=biasM, scale=OIM_SCALAR,
                                     accum_out=ssb[:, m:m + 1])

            sacc_cur = {}
            ones_lag = []     # software-pipelined (w, t, eb_t) ones-matmuls

            def flush_ones(keep):
                # 3 batch-windows share one PSUM bank, at partition offsets
                # 0/32/64; one full-bank copy drains all three row-sums.
                while len(ones_lag) > keep:
                    w, t, eb_t = ones_lag.pop(0)
                    win = t >= NQT
                    tag = "win" if win else "raw"
                    row = 32 * (w % 3)
                    nc.tensor.matmul(sacc_cur[tag][row:row + 1, :],
                                     ones16, eb_t,
                                     start=(t == (NQT if win else 0)),
                                     stop=(t == (NT - 1 if win else NQT - 1)))
                    if ((w % 3 == 2 or w == NBW - 1)
                            and t == (NT - 1 if win else NQT - 1)):
                        k = (3 if win else 0) + w // 3
                        nc.vector.tensor_scalar_mul(
                            out=srow_sb[:, k, :], in0=sacc_cur[tag],
                            scalar1=1.0)
                        nc.sync.dma_start(out=srow_d[:, k, :],
                                          in_=srow_sb[:, k, :])

            def emit_Bdr(w, t):
                win = t >= NQT
                tag = "win" if win else "raw"
                if t in (0, NQT) and w % 3 == 0:
                    flush_ones(0)   # finish this tag's previous accumulator
                    sacc_cur[tag] = psum_s.tile([128, BW], f32,
                                                tag="sacc",
                                                name=f"sacc_{tag}{w}")
                lhs = uembT8 if win else embT8
                toff = (t - NQT) if win else t
                pb_t = psum_b.tile([128, 512], f32, tag="pb", name=f"pb{w}_{t}")
                for h in range(2):
                    for c in range(KD2):
                        nc.tensor.matmul(
                            pb_t[:, h * 256:(h + 1) * 256],
                            lhs[:, 2 * c:2 * c + 2, toff * 128:(toff + 1) * 128],
                            xnT8[:, 2 * c:2 * c + 2,
                                 w * BW + h * 256:w * BW + (h + 1) * 256],
                            start=(c == 0), stop=(c == KD2 - 1),
                            perf_mode=DR)
                if e_B(w, t):
                    ii = eap.tile([128, 512], mybir.dt.int16, tag="ii",
                                  name=f"ii{w}_{t}")
                    nc.vector.tensor_scalar(
                        out=ii, in0=pb_t, scalar1=A16 * OIM_SCALAR,
                        scalar2=SB16 - A16 * M, op0=OP.mult, op1=OP.add)
                    eb_t = ii.bitcast(bf16)
                else:
                    eb_t = eap.tile([128, 512], bf16, tag="ea", name=f"eb{w}_{t}")
                    nc.scalar.activation(out=eb_t, in_=pb_t, func=AF.Exp,
                                         bias=biasM, scale=OIM_SCALAR)
                ones_lag.append((w, t, eb_t))
                flush_ones(5)

            def emit_means_chunk(pm, mu, dc, c16):
                # NOTE: the dc=0 / dc=1 groups share a PSUM bank; a group's
                # `start` pending-zeroes the whole bank, so the groups must
                # run sequentially (all of dc=0, then all of dc=1).
                nc.tensor.matmul(
                    pm[:, dc * 256:(dc + 1) * 256],
                    mask8[:, c16, :, mu * 128:(mu + 1) * 128],
                    x8b[:, c16, :, dc * 256:(dc + 1) * 256],
                    start=(c16 == 0), stop=(c16 == NC16 - 1),
                    perf_mode=DR)

            def emit_means_fin():
                # normalize: rin = rsqrt(sum of squares), Quake + 2 Newton
                ssq2 = small.tile([128, 2], f32, tag="ssq2")
                for mu in range(2):
                    sq = scrp.tile([128, D], f32, tag="sq", name=f"sqm{mu}")
                    nc.vector.scalar_tensor_tensor(
                        out=sq, in0=uraw[:, mu, :], scalar=1.0,
                        in1=uraw[:, mu, :], op0=OP.mult, op1=OP.mult,
                        accum_out=ssq2[:, mu:mu + 1])
                nc.vector.tensor_scalar_max(out=ssq2, in0=ssq2, scalar1=1e-24)
                ish = small.tile([128, 2], i32, tag="ish")
                nc.vector.tensor_scalar(out=ish, in0=ssq2.bitcast(i32),
                                        scalar1=1, scalar2=None,
                                        op0=OP.arith_shift_right)
                nc.vector.tensor_scalar(out=ish, in0=ish,
                                        scalar1=-1, scalar2=0x5F3759DF,
                                        op0=OP.mult, op1=OP.add)
                y = ish.bitcast(f32)
                t1 = small.tile([128, 2], f32, tag="t1")
                for _ in range(2):  # Newton: y *= 1.5 - 0.5*ssq*y*y
                    nc.vector.tensor_tensor(out=t1, in0=y, in1=y, op=OP.mult)
                    nc.vector.tensor_tensor(out=t1, in0=t1, in1=ssq2, op=OP.mult)
                    nc.vector.tensor_scalar(out=t1, in0=t1, scalar1=-0.5,
                                            scalar2=1.5, op0=OP.mult, op1=OP.add)
                    nc.vector.tensor_tensor(out=y, in0=y, in1=t1, op=OP.mult)
                for mu in range(2):
                    nc.vector.tensor_scalar_mul(
                        out=uemb_n[:, mu, :], in0=uraw[:, mu, :],
                        scalar1=y[:, mu:mu + 1])
                for mu in range(2):
                    for kd in range(KD):
                        pst = psum_m.tile([128, D], f32, tag="pm",
                                          name=f"pst{mu}_{kd}")
                        nc.tensor.transpose(
                            pst[:, :128], uemb_n[:, mu, kd * 128:(kd + 1) * 128],
                            ident)
                        nc.vector.tensor_scalar_mul(
                            out=uembT8[:, kd, mu * 128:(mu + 1) * 128],
                            in0=pst[:, :128], scalar1=1.0)
                nc.sync.dma_start(out=uout_d, in_=uembT8)

            with (
                tc.tile_pool(name="psum_a", bufs=1, space="PSUM") as psum_a,
                tc.tile_pool(name="psum_b", bufs=4, space="PSUM") as psum_b,
                tc.tile_pool(name="psum_s", bufs=1, space="PSUM") as psum_s,
                tc.tile_pool(name="psum_m", bufs=1, space="PSUM") as psum_m,
            ):
                # PE warmup during the DMA window (also ramps the p-state)
                for k in range(12):
                    pst = psum_m.tile([128, D], f32, tag="pm", name=f"wu{k}")
                    nc.tensor.transpose(pst[:, :128], ident, ident)

                # B units: raw tiles first, window tiles once means are done
                raw = [(w, t) for w in range(NBW) for t in range(NQT)]
                win = [(w, t) for w in range(NBW) for t in range(NQT, NT)]
                bts = raw + win
                bi = 0
                pm = None
                for m in range(MT):
                    emit_A(m)
                    if m in (6, 10):
                        mu = 0 if m == 6 else 1
                        pm = psum_m.tile([128, D], f32, tag="pm",
                                         name=f"pm{mu}")
                    if 6 <= m <= 13:
                        mu = 0 if m <= 9 else 1
                        s = m - 6 - 4 * mu          # 0..3 within this mu
                        dc = s // 2                 # dc=0 fully, then dc=1
                        for c16 in range(8 * (s % 2), 8 * (s % 2) + 8):
                            emit_means_chunk(pm, mu, dc, c16)
                        if m in (9, 13):   # drain this mu's accumulator
                            nc.scalar.copy(out=uraw[:, mu, :], in_=pm)
                    if m == 14:
                        emit_means_fin()
                    want = ((m + 1) * len(bts) + MT - 1) // MT
                    while bi < min(want, len(bts)):
                        emit_Bdr(*bts[bi])
                        bi += 1
                while bi < len(bts):
                    emit_Bdr(*bts[bi])
                    bi += 1
                flush_ones(0)

            nc.sync.dma_start(out=sume_d, in_=ssb)

    nc.compile()
    return nc


def _host_bookkeeping(labels, label_cq, header_cq):
    """Mirror the reference's integer-only queue-update semantics."""
    labels = np.asarray(labels).astype(np.int64)
    lab = np.asarray(label_cq).astype(np.int64).copy()
    h0 = int(np.asarray(header_cq))

    uq = np.unique(labels)
    if uq.size < U:
        uniq = np.concatenate([uq, np.full(U - uq.size, uq.min(), np.int64)])
    else:
        uniq = uq[:U]

    emb_src = np.full(Q, -1, np.int64)   # >=0: row u of uniq means; -1: original
    h = h0 % Q
    for u in range(U):
        y = uniq[u]
        m = lab == y
        i = int(np.argmax(m)) if m.any() else 0
        inval = bool(m.any()) and (i != h)
        emb_src[h] = u
        lab[h] = y
        if inval:
            lab[i] = IGNORE
        h = (h + 1) % Q

    good = lab != IGNORE
    goodidx = np.flatnonzero(good)
    gl = lab[goodidx]
    vals, first = np.unique(gl, return_index=True)
    pos = np.searchsorted(vals, labels)
    assert np.all(vals[np.clip(pos, 0, vals.size - 1)] == labels), \
        "batch label missing from queue"
    xe = goodidx[first[pos]]
    return uniq, emb_src, good, xe


def _zero_counts(keepNW):
    """Per-batch-row counts of zeroed columns, split by exp engine."""
    zero_col = ~keepNW.reshape(N_CORES, QS)
    zb = zero_col[:, :QB].reshape(N_CORES, NQT, 128).sum(axis=2)   # [core, t]
    za = zero_col[:, QB:].sum(axis=1)                              # [core]
    n_s = np.zeros(B, np.int64)
    n_tot = int(zero_col.sum())
    # A units are always exact-exp (ACT); B raw tiles follow e_B
    for w in range(NBW):
        cnt = sum(int(zb[c, t]) for c in range(N_CORES)
                  for t in range(NQT) if e_B(w, t))
        n_s[w * BW:(w + 1) * BW] += cnt
    return n_tot - n_s, n_s      # (n_act[b], n_schr[b])


def _prepare(inputs, labels, emb_cq, label_cq, header_cq):
    import ml_dtypes
    f8 = ml_dtypes.float8_e4m3

    inputs = np.ascontiguousarray(np.asarray(inputs, np.float32))
    emb_cq = np.ascontiguousarray(np.asarray(emb_cq, np.float32))
    labels = np.asarray(labels).astype(np.int64)

    uniq, emb_src, good, xe = _host_bookkeeping(labels, label_cq, header_cq)

    keepNW = good & (emb_src < 0)
    n_zero = _zero_counts(keepNW)

    x64 = inputs.astype(np.float64)
    xn = x64 / np.maximum(np.linalg.norm(x64, axis=1, keepdims=True), 1e-12)
    xn8 = xn.astype(f8)
    x8 = inputs.astype(f8)
    embz = emb_cq.copy()
    embz[~keepNW] = 0.0
    emb8 = embz.astype(f8)

    mx = float(np.linalg.norm(xn8.astype(np.float64), axis=1).max())
    me = float(np.linalg.norm(emb8.astype(np.float64), axis=1).max())
    M = OIM_SCALAR * max(1.0, mx * me, mx * 1.1) * 1.02

    w_idx = emb_src[xe]                           # -1 for non-window targets
    extra = np.flatnonzero(w_idx < 0)             # handled on host (rare/none)

    def dmajor(a8):  # [R, D] fp8 -> [128, KD, R]
        return np.ascontiguousarray(a8.T.reshape(KD, 128, -1).transpose(1, 0, 2))

    def bpair(a8, cols):  # [B, cols] fp8 -> [128, NC16, 2, cols]
        return np.ascontiguousarray(
            a8.reshape(NC16, 2, 128, cols).transpose(2, 0, 1, 3))

    mask8 = (labels[:, None] == uniq[None, :]).astype(f8)

    base = {
        "xnT8": dmajor(xn8),
        "x8b": bpair(x8, D),
        "mask8": bpair(mask8, U),
    }
    in_maps = []
    for c in range(N_CORES):
        in_maps.append({
            **base,
            "embT8": dmajor(emb8[c * QS:(c + 1) * QS]),
        })
    return M, in_maps, n_zero, extra, xe, w_idx, xn8


def _schr_val(z):
    """bf16 bit-trick exp value the device produces for logit-sum z."""
    import ml_dtypes
    i = np.rint(A16 * z + SB16).astype(np.int16)
    return float(i.view(ml_dtypes.bfloat16).astype(np.float64))


def _combine(res_list, M, n_zero, extra, xe, w_idx, xn8, inputs, emb_cq):
    n_zero_a, n_zero_s = n_zero
    S = np.zeros(B, np.float64)

    def brow(sr, off):      # sections off..off+2, 3 windows per bank
        out = np.zeros(B, np.float64)
        for w in range(NBW):
            sec, row = off + w // 3, 32 * (w % 3)
            out[w * BW:(w + 1) * BW] = sr[row, sec, :]
        return out

    for r in res_list:
        S += r["sume"].astype(np.float64).T.reshape(B)
        S += brow(r["srow"].astype(np.float64), 0)
    S += brow(res_list[0]["srow"].astype(np.float64), 3)
    S -= n_zero_a * np.exp(-M) + n_zero_s * _schr_val(np.float64(-M))

    # target cosine: bit-exact fp8 gather-dot using the device's uembT8
    uT8 = res_list[0]["uout"]                       # [128, KD, U] fp8
    u8 = np.ascontiguousarray(
        uT8.transpose(1, 0, 2).reshape(D, U).T).astype(np.float32)  # [U, D]
    xw = xn8.astype(np.float32)[np.arange(B)]
    t_cos = np.einsum("bd,bd->b", xw,
                      u8[np.clip(w_idx, 0, U - 1)]).astype(np.float64)
    if extra.size:  # targets pointing at original (non-window) queue rows
        xb = np.asarray(inputs, np.float64)[extra]
        xb /= np.maximum(np.linalg.norm(xb, axis=1, keepdims=True), 1e-12)
        eb = np.asarray(emb_cq, np.float64)[xe[extra]]
        t_cos[extra] = (xb * eb).sum(axis=1)

    loss = np.mean(M + np.log(S) - OIM_SCALAR * t_cos)
    return np.array(loss, dtype=np.float32)


def kernel(inputs, labels, emb_cq, label_cq, age_cq, header_cq):
    from concourse.bass_utils import run_bass_kernel_spmd

    M, in_maps, n_zero, extra, xe, w_idx, xn8 = _prepare(
        inputs, labels, emb_cq, label_cq, header_cq)
    key = round(M, 9)
    if key not in _PROG_CACHE:
        _PROG_CACHE[key] = _build_program(M)
    nc = _PROG_CACHE[key]

    res = run_bass_kernel_spmd(nc, in_maps, core_ids=list(range(N_CORES)))
    return _combine(res.results, M, n_zero, extra, xe, w_idx, xn8,
                    inputs, emb_cq)
